# revision 18
# baseline (speedup 1.0000x reference)
"""CRF NLL loss kernel for Trainium2 (8 NeuronCores, data-parallel over batch).

Math: the forward recurrence alpha_t = LSE_j(alpha_{t-1,j} + trans[j,k]) + emit_t
is computed in probability space:  P_t = Eemit_t * (Etrans^T @ P_{t-1})
with per-step normalizers d_t = mean_b LSE_k(emit[t,b,:]) (host-precomputed)
keeping P in f32 range.

Meet-in-the-middle: a forward chain produces P_255 (255 serial steps) while an
independent backward chain runs from t=511 down to t=256 producing
X_256 = Ehat_256 * (M @ X_257), so logZ_b = log(P_255 . (M @ X_256)) + D[end_b].
Variable sequence lengths are handled exactly by rewriting the backward
emission columns on the host: beyond a sequence's end the state is held at the
Perron vector r of M (each padded step scales by 1/lambda), and the single
boundary step uses v/r with v = M^{-1} w (w = exp(etrans)) which maps r -> w.
This halves the serial-latency-bound span (the dominant cost) vs a single
forward sweep.  The gold-path score is pure gather work, done on host in f64.
"""

import numpy as np

import concourse.bacc as bacc
import concourse.mybir as mybir
import concourse.tile as tile
from concourse.bass_utils import run_bass_kernel_spmd

T, B, N = 512, 256, 128
NCORES = 8
BL = B // NCORES          # 32 sequences per core
NSTEP = 255               # serial steps per chain (fwd: t=1..255, bwd: t=510..256)
CHUNK = 32                # emit steps per DMA chunk
CHUNK0 = 8                # first chunk is small so the chains start early
ELT_ENGINE = "vector"     # "gpsimd" (Pool) or "vector" (DVE)

LAST_RESULTS = None       # BassKernelResults of the last run (for profiling)

_compiled = {}


def _build_nc():
    nc = bacc.Bacc("TRN2", target_bir_lowering=False, debug=False,
                   num_devices=NCORES)
    f32 = mybir.dt.float32
    # initf/initb pack [state0 | weights | first emission chunk] so each
    # chain's critical lead-in needs a single HWDGE generation slot (the
    # descriptor generator is shared across queues and serializes at ~630ns).
    IW = BL + N + CHUNK0 * BL
    initf = nc.dram_tensor("initf", [N, IW], f32, kind="ExternalInput")
    initb = nc.dram_tensor("initb", [N, IW], f32, kind="ExternalInput")
    efwd = nc.dram_tensor("efwd", [N, (NSTEP - CHUNK0) * BL], f32,
                          kind="ExternalInput")
    ebwd = nc.dram_tensor("ebwd", [N, (NSTEP - CHUNK0) * BL], f32,
                          kind="ExternalInput")
    pqout = nc.dram_tensor("pqout", [N, 2 * BL], f32, kind="ExternalOutput")

    elt = getattr(nc, ELT_ENGINE).tensor_tensor

    # step -> (chunk index, offset) with a small first chunk so the chains
    # can start while the bulk of the emissions is still in flight
    bounds = [0, CHUNK0]
    while bounds[-1] < NSTEP:
        bounds.append(min(bounds[-1] + CHUNK, NSTEP))
    n_chunks = len(bounds) - 1

    with tile.TileContext(nc) as tc:
        with (
            tc.tile_pool(name="const", bufs=1) as cpool,
            tc.tile_pool(name="emitf", bufs=n_chunks) as efpool,
            tc.tile_pool(name="emitb", bufs=n_chunks) as ebpool,
            tc.tile_pool(name="pstate", bufs=4) as fppool,
            tc.tile_pool(name="xstate", bufs=4) as bppool,
            tc.tile_pool(name="psumf", bufs=3, space="PSUM") as fspool,
            tc.tile_pool(name="psumb", bufs=3, space="PSUM") as bspool,
            tc.tile_pool(name="outp", bufs=1) as opool,
        ):
            # One merged critical DMA per chain on separate queues.
            IW = BL + N + CHUNK0 * BL
            tF0 = cpool.tile([N, IW], f32, tag="initf")
            nc.sync.dma_start(tF0[:], initf[:])
            tB0 = cpool.tile([N, IW], f32, tag="initb")
            nc.gpsimd.dma_start(tB0[:], initb[:])
            p_cur = tF0[:, 0:BL]
            mF = tF0[:, BL:BL + N]
            x_cur = tB0[:, 0:BL]
            mB = tB0[:, BL:BL + N]

            fch = [tF0] * n_chunks
            bch = [tB0] * n_chunks
            fbase = [0] * n_chunks
            fbase[0] = BL + N

            for c in range(1, n_chunks):
                lo = (bounds[c] - CHUNK0) * BL
                hi = (bounds[c + 1] - CHUNK0) * BL
                tF = efpool.tile([N, CHUNK * BL], f32, tag="ef")
                nc.sync.dma_start(tF[:, :hi - lo], efwd[:, lo:hi])
                fch[c] = tF
                tB = ebpool.tile([N, CHUNK * BL], f32, tag="eb")
                nc.scalar.dma_start(tB[:, :hi - lo], ebwd[:, lo:hi])
                bch[c] = tB

            out_t = opool.tile([N, 2 * BL], f32, tag="pq")

            for s in range(NSTEP):
                c = next(i for i in range(n_chunks) if bounds[i + 1] > s)
                off = s - bounds[c]
                sl = slice(fbase[c] + off * BL, fbase[c] + (off + 1) * BL)
                last = s == NSTEP - 1

                sF = fspool.tile([N, BL], f32, tag="sf")
                nc.tensor.matmul(sF[:], mF, p_cur, start=True, stop=True)
                if last:
                    p_dst = out_t[:, 0:BL]
                else:
                    p_new = fppool.tile([N, BL], f32, tag="p")
                    p_dst = p_new[:]
                elt(p_dst, sF[:], fch[c][:, sl], mybir.AluOpType.mult)
                p_cur = p_dst

                sB = bspool.tile([N, BL], f32, tag="sb")
                nc.tensor.matmul(sB[:], mB, x_cur, start=True, stop=True)
                if last:
                    x_dst = out_t[:, BL:2 * BL]
                else:
                    x_new = bppool.tile([N, BL], f32, tag="x")
                    x_dst = x_new[:]
                elt(x_dst, sB[:], bch[c][:, sl], mybir.AluOpType.mult)
                x_cur = x_dst

            nc.sync.dma_start(pqout[:], out_t[:])
    nc.compile()
    return nc


def kernel(emit, target, mask, trans, strans, etrans):
    global LAST_RESULTS
    emit = np.asarray(emit, dtype=np.float32)
    target = np.asarray(target, dtype=np.int32)
    mask = np.asarray(mask)
    trans = np.asarray(trans, dtype=np.float32)
    strans = np.asarray(strans, dtype=np.float32)
    etrans = np.asarray(etrans, dtype=np.float32)

    # --- host preprocessing ---
    # per-step normalizer d_t (f64): mean over batch of LSE_k emit[t]
    e64 = emit.astype(np.float64)
    m_t = e64.max(axis=2, keepdims=True)
    lse = (m_t[..., 0] + np.log(np.exp(e64 - m_t).sum(axis=2)))  # [T,B]
    d = lse.mean(axis=1)                                         # [T]
    d[0] = 0.0
    D = np.cumsum(d)                                             # [T]

    eemn = np.exp(e64 - d[:, None, None])                        # [T,B,N]
    M64 = np.exp(trans.astype(np.float64))                       # [N,N] (j,k)
    w64 = np.exp(etrans.astype(np.float64))                      # [N]

    # Perron vector/value of M64 and v = M^{-1} w for the backward padding
    r = np.ones(N, dtype=np.float64)
    for _ in range(60):
        r = M64 @ r
        r /= r.mean()
    lam = float((r @ (M64 @ r)) / (r @ r))
    v = np.linalg.solve(M64, w64)                                # [N]

    L = mask.astype(np.int64).sum(axis=0)                        # [B]
    ends = L - 1                                                 # in [255, 511]

    # forward: P0 and E'_t for t=1..255, laid out [N, t, B]
    P0 = np.exp(strans[None, :].astype(np.float64) + e64[0]).T   # [N,B]
    EF = np.ascontiguousarray(eemn[1:NSTEP + 1].transpose(2, 0, 1))  # [N,255,B]

    # backward emissions Ehat_t for t=256..510 (indexed i=t-256) + init X_511
    tt = np.arange(256, 511)                                     # [255]
    EB = eemn[256:511].transpose(2, 0, 1).copy()                 # [N,255,B]
    pad = tt[None, :] > L[:, None]                               # [B,255] t > L_b
    bnd = tt[None, :] == L[:, None]                              # [B,255] t == L_b
    padT = pad.T[None, :, :]                                     # [1,255,B]
    bndT = bnd.T[None, :, :]
    EB = np.where(padT, 1.0 / lam, EB)
    EB = np.where(bndT, (v / r)[:, None, None], EB)
    # consumption order: step s consumes Ehat_{510-s}  (i = 254-s)
    EBc = np.ascontiguousarray(EB[:, ::-1, :])                   # [N,255,B]

    X0 = np.empty((N, B), dtype=np.float64)                      # X_511 per col
    full = L == T                                                # L_b = 512
    last = L == T - 1                                            # L_b = 511
    rest = ~(full | last)
    if full.any():
        X0[:, full] = (eemn[511, full, :] * w64[None, :]).T
    if last.any():
        X0[:, last] = v[:, None]
    if rest.any():
        X0[:, rest] = (r / lam)[:, None]

    M32 = M64.astype(np.float32)
    MT32 = np.ascontiguousarray(M64.T).astype(np.float32)
    in_maps = []
    for c in range(NCORES):
        sl = slice(c * BL, (c + 1) * BL)
        ef = EF[:, :, sl].reshape(N, NSTEP * BL).astype(np.float32)
        eb = EBc[:, :, sl].reshape(N, NSTEP * BL).astype(np.float32)
        in_maps.append({
            "initf": np.ascontiguousarray(np.concatenate(
                [P0[:, sl].astype(np.float32), M32, ef[:, :CHUNK0 * BL]],
                axis=1)),
            "initb": np.ascontiguousarray(np.concatenate(
                [X0[:, sl].astype(np.float32), MT32, eb[:, :CHUNK0 * BL]],
                axis=1)),
            "efwd": np.ascontiguousarray(ef[:, CHUNK0 * BL:]),
            "ebwd": np.ascontiguousarray(eb[:, CHUNK0 * BL:]),
        })

    if "nc" not in _compiled:
        _compiled["nc"] = _build_nc()
    nc = _compiled["nc"]

    res = run_bass_kernel_spmd(nc, in_maps, core_ids=list(range(NCORES)))
    LAST_RESULTS = res

    # --- host postprocessing ---
    P255 = np.concatenate(
        [res.results[c]["pqout"][:, :BL].astype(np.float64)
         for c in range(NCORES)], axis=1)                        # [N,B]
    X256 = np.concatenate(
        [res.results[c]["pqout"][:, BL:].astype(np.float64)
         for c in range(NCORES)], axis=1)                        # [N,B]
    Q255 = M64 @ X256                                            # [N,B]
    dot = (P255 * Q255).sum(axis=0)                              # [B]
    logZ = (np.log(dot) + D[ends]).sum()

    # gold score (f64, mirrors reference)
    tb = np.arange(B)
    emit_sc = np.take_along_axis(e64, target[:, :, None].astype(np.int64),
                                 axis=2)[..., 0]                 # [T,B]
    trans_sc = trans.astype(np.float64)[target[:-1], target[1:]]  # [T-1,B]
    scores = emit_sc.copy()
    scores[1:] += trans_sc
    score = np.where(mask, scores, 0.0).sum()
    score += strans.astype(np.float64)[target[0]].sum()
    score += etrans.astype(np.float64)[target[ends, tb]].sum()

    loss = (logZ - score) / B
    return np.float32(loss)


# revision 24
# speedup vs baseline: 1.4585x; 1.4585x over previous
"""CRF NLL loss kernel for Trainium2 (8 NeuronCores, data-parallel over batch).

Math: the forward recurrence alpha_t = LSE_j(alpha_{t-1,j} + trans[j,k]) + emit_t
is computed in probability space:  P_t = Eemit_t * (Etrans^T @ P_{t-1})
with per-step normalizers d_t = mean_b LSE_k(emit[t,b,:]) (host-precomputed).

Parallel-segment decomposition: products of positive matrices contract in the
Hilbert projective metric (diagonal emission scalings are isometries), so a
chain started from an arbitrary positive vector converges to the true state's
DIRECTION within ~16 steps; magnitudes differ by one scalar per column which
the host recovers by stitching shipped boundary states.  This turns the
T-step serial recurrence into 6 concurrent latency-bound streams:
  forward:  F1 (t 1..109), F2 (94..203, 16-step burn-in), F3 (188..255, burn)
  backward: B1 (t 510..383), B2 (399..319, burn), B3 (335..256, burn)
Backward runs in X-space X_t = Ehat_t * (M @ X_{t+1}); variable sequence ends
are emission-rewrites on the host: padded steps hold the Perron vector r of
M = exp(trans) (scalar 1/lambda per step) and the single boundary step uses
v/r, v = M^{-1} exp(etrans), mapping r -> w exactly.
Engine placement: F1,F2,B1 elementwise on DVE (551ns/step chain latency);
F3,B2,B3 use Act-copy(PSUM->SBUF)+GPSIMD-mult (884ns/step) since the
Activation engine has no tensor_tensor and GPSIMD cannot read PSUM.
Host computes logZ_b = log(P(255) . M X(256)) + stitch scalars + D[end_b];
the gold-path score is pure gather work, done on host in f64.
"""

import numpy as np
import ml_dtypes

import concourse.bacc as bacc
import concourse.mybir as mybir
import concourse.tile as tile
from concourse.bass_utils import run_bass_kernel_spmd

T, B, N = 512, 256, 128
NCORES = 8
BL = B // NCORES          # 32 sequences per core
K = 16                    # burn-in steps for non-initial streams
M1, M2 = 109, 203         # forward boundaries
N1, N2 = 383, 319         # backward boundaries
CHUNK = 32                # emission steps per DMA chunk
F0 = 8                    # first-chunk steps folded into init DMA (F1/B1)

BF = ml_dtypes.bfloat16

# name, engine, direction, consumed t-list, ships {step_idx: slot}, init
def _mk_streams():
    f1 = list(range(1, M1 + 1))
    f2 = list(range(M1 - K + 1, M2 + 1))
    f3 = list(range(M2 - K + 1, 256))
    b1 = list(range(510, N1 - 1, -1))
    b2 = list(range(N1 + K, N2 - 1, -1))
    b3 = list(range(N2 + K, 255, -1))
    return [
        dict(name="F1", eng="dve", dr="F", ts=f1, ships={len(f1) - 1: 0},
             init="dma", first=F0, period=551, phase=0),
        dict(name="F2", eng="dve", dr="F", ts=f2,
             ships={K - 1: 1, len(f2) - 1: 2}, init="ones", first=K,
             period=551, phase=184),
        dict(name="F3", eng="hyb", dr="F", ts=f3,
             ships={K - 1: 3, len(f3) - 1: 4}, init="ones", first=K,
             period=884, phase=90),
        dict(name="B1", eng="dve", dr="B", ts=b1, ships={len(b1) - 1: 5},
             init="dma", first=F0, period=551, phase=367),
        dict(name="B2", eng="hyb", dr="B", ts=b2,
             ships={K: 6, len(b2) - 1: 7}, init="ones", first=K,
             period=884, phase=385),
        dict(name="B3", eng="hyb", dr="B", ts=b3,
             ships={K: 8, len(b3) - 1: 9}, init="ones", first=K,
             period=884, phase=680),
    ]

STREAMS = _mk_streams()
NSHIP = 10

LAST_RESULTS = None       # BassKernelResults of the last run (for profiling)

_compiled = {}


def _build_nc():
    nc = bacc.Bacc("TRN2", target_bir_lowering=False, debug=False,
                   num_devices=NCORES)
    f32 = mybir.dt.float32
    bf16 = mybir.dt.bfloat16

    # merged critical loads: [state0 | weights | first chunks of same-direction
    # streams] -> one HWDGE generation slot each (the generator is shared
    # across queues and serializes at ~630ns)
    IWF = BL + N + F0 * BL + K * BL + K * BL
    initf = nc.dram_tensor("initf", [N, IWF], bf16, kind="ExternalInput")
    initb = nc.dram_tensor("initb", [N, IWF], bf16, kind="ExternalInput")
    emd = {}
    for st in STREAMS:
        w = (len(st["ts"]) - st["first"]) * BL
        emd[st["name"]] = nc.dram_tensor("em" + st["name"], [N, max(w, BL)],
                                         bf16, kind="ExternalInput")
    ships = nc.dram_tensor("ships", [N, NSHIP * BL], bf16,
                           kind="ExternalOutput")

    with tile.TileContext(nc) as tc:
        with (
            tc.tile_pool(name="const", bufs=1) as cpool,
            tc.tile_pool(name="emit", bufs=max(
                (len(st["ts"]) - st["first"] + CHUNK - 1) // CHUNK
                for st in STREAMS)) as epool,
            tc.tile_pool(name="state", bufs=3) as spool,
            tc.tile_pool(name="tmp", bufs=3) as tpool,
            tc.tile_pool(name="ps", bufs=1, space="PSUM") as pspool,
        ):
            tF0 = cpool.tile([N, IWF], bf16, tag="initf")
            nc.sync.dma_start(tF0[:], initf[:])
            tB0 = cpool.tile([N, IWF], bf16, tag="initb")
            nc.gpsimd.dma_start(tB0[:], initb[:])

            mF = tF0[:, BL:BL + N]
            mB = tB0[:, BL:BL + N]

            # ones init for burn-in streams
            t_ones = cpool.tile([N, BL], bf16, tag="ones")
            nc.vector.memset(t_ones[:], 1.0)

            # per-stream runtime state
        # layout of first-chunk region inside init tiles:
            #   [state0 | M | first(F1 or B1) | first(F2/B2) | first(F3/B3)]
            offs = {
                "F1": BL + N, "B1": BL + N,
                "F2": BL + N + F0 * BL, "B2": BL + N + F0 * BL,
                "F3": BL + N + (F0 + K) * BL, "B3": BL + N + (F0 + K) * BL,
            }
            rt = {}
            chunk_reqs = []   # (need_vtime, stream, chunk lo, hi)
            for st in STREAMS:
                nm = st["name"]
                it = tF0 if st["dr"] == "F" else tB0
                n_steps = len(st["ts"])
                # emission AP per step (first chunk lives in the init tile)
                em_ap = []
                for k in range(st["first"]):
                    em_ap.append((it, offs[nm] + k * BL))
                n_rest = n_steps - st["first"]
                n_ch = (n_rest + CHUNK - 1) // CHUNK
                for c in range(n_ch):
                    lo = c * CHUNK * BL
                    hi = min(n_rest, (c + 1) * CHUNK) * BL
                    need = st["phase"] + (st["first"] + c * CHUNK) * st["period"]
                    chunk_reqs.append((need, nm, lo, hi))
                cur = it[:, 0:BL] if st["init"] == "dma" else t_ones[:]
                rt[nm] = dict(st=st, cur=cur, em=em_ap,
                              m=mF if st["dr"] == "F" else mB)

            # all chunk DMAs on SP, ordered by first-consumption time, so the
            # Activation sequencer serves only the hybrid-stream copies
            chunk_reqs.sort()
            for _, nm, lo, hi in chunk_reqs:
                ch_t = epool.tile([N, CHUNK * BL], bf16, tag="em" + nm)
                nc.sync.dma_start(ch_t[:, :hi - lo], emd[nm][:, lo:hi])
                for k in range((hi - lo) // BL):
                    rt[nm]["em"].append((ch_t, k * BL))

            # ship destination tiles (dedicated, never recycled)
            ship_t = []
            for i in range(NSHIP):
                sh = cpool.tile([N, BL], bf16, tag=f"ship{i}")
                ship_t.append(sh)

            # static virtual-time schedule
            events = []
            for st in STREAMS:
                for k in range(len(st["ts"])):
                    events.append((st["phase"] + k * st["period"],
                                   st["name"], k))
            events.sort()

            for _, nm, k in events:
                r = rt[nm]
                st = r["st"]
                ps_t = pspool.tile([N, BL], f32, tag="ps" + nm)
                nc.tensor.matmul(ps_t[:], r["m"], r["cur"],
                                 start=True, stop=True)
                ch_t, off = r["em"][k]
                em = ch_t[:, off:off + BL]
                slot = st["ships"].get(k)
                if slot is None:
                    o_t = spool.tile([N, BL], bf16, tag="s" + nm)
                    out = o_t[:]
                else:
                    out = ship_t[slot][:]
                if st["eng"] == "dve":
                    nc.vector.tensor_tensor(out, ps_t[:], em,
                                            mybir.AluOpType.mult)
                else:
                    tm_t = tpool.tile([N, BL], bf16, tag="t" + nm)
                    nc.scalar.copy(tm_t[:], ps_t[:])
                    nc.gpsimd.tensor_tensor(out, tm_t[:], em,
                                            mybir.AluOpType.mult)
                r["cur"] = out
                if slot is not None:
                    q = nc.sync if st["dr"] == "F" else nc.scalar
                    q.dma_start(ships[:, slot * BL:(slot + 1) * BL], out)
    nc.compile()
    return nc


def kernel(emit, target, mask, trans, strans, etrans):
    global LAST_RESULTS
    emit = np.asarray(emit, dtype=np.float32)
    target = np.asarray(target, dtype=np.int32)
    mask = np.asarray(mask)
    trans = np.asarray(trans, dtype=np.float32)
    strans = np.asarray(strans, dtype=np.float32)
    etrans = np.asarray(etrans, dtype=np.float32)

    # --- host preprocessing ---
    e64 = emit.astype(np.float64)
    m_t = e64.max(axis=2, keepdims=True)
    lse = (m_t[..., 0] + np.log(np.exp(e64 - m_t).sum(axis=2)))  # [T,B]
    d = lse.mean(axis=1)
    d[0] = 0.0
    D = np.cumsum(d)

    eemn = np.exp(e64 - d[:, None, None])                        # [T,B,N]
    M64 = np.exp(trans.astype(np.float64))                       # [N,N] (j,k)
    w64 = np.exp(etrans.astype(np.float64))

    r = np.ones(N, dtype=np.float64)
    for _ in range(60):
        r = M64 @ r
        r /= r.mean()
    lam = float((r @ (M64 @ r)) / (r @ r))
    v = np.linalg.solve(M64, w64)

    L = mask.astype(np.int64).sum(axis=0)
    ends = L - 1

    P0 = np.exp(strans[None, :].astype(np.float64) + e64[0]).T   # [N,B]

    # backward emissions Ehat_t for t=256..510 indexed [N, t, B]
    tt = np.arange(256, 511)
    EB = eemn[256:511].transpose(2, 0, 1).copy()                 # [N,255,B]
    pad = (tt[None, :] > L[:, None]).T[None, :, :]
    bnd = (tt[None, :] == L[:, None]).T[None, :, :]
    EB = np.where(pad, 1.0 / lam, EB)
    EB = np.where(bnd, (v / r)[:, None, None], EB)

    def em_at(t):
        # [N, B] emission consumed at step t (fwd E'_t or bwd Ehat_t)
        if t <= 255:
            return eemn[t].T
        return EB[:, t - 256, :]

    X0 = np.empty((N, B), dtype=np.float64)
    full = L == T
    last = L == T - 1
    rest = ~(full | last)
    if full.any():
        X0[:, full] = (eemn[511, full, :] * w64[None, :]).T
    if last.any():
        X0[:, last] = v[:, None]
    if rest.any():
        X0[:, rest] = (r / lam)[:, None]

    # per-stream emission arrays in consumption order
    em_all = {}
    for st in STREAMS:
        em_all[st["name"]] = np.stack([em_at(t) for t in st["ts"]],
                                      axis=1)                    # [N,steps,B]

    in_maps = []
    Mbf = M64.astype(BF)
    MTbf = np.ascontiguousarray(M64.T).astype(BF)
    for c in range(NCORES):
        sl = slice(c * BL, (c + 1) * BL)
        im = {}
        for drn, s0, mm, first_sts in (
                ("initf", P0[:, sl], Mbf, ("F1", "F2", "F3")),
                ("initb", X0[:, sl], MTbf, ("B1", "B2", "B3"))):
            parts = [s0.astype(BF), mm]
            for nm in first_sts:
                st = next(s for s in STREAMS if s["name"] == nm)
                parts.append(em_all[nm][:, :st["first"], sl]
                             .reshape(N, -1).astype(BF))
            im[drn] = np.ascontiguousarray(np.concatenate(parts, axis=1))
        for st in STREAMS:
            nm = st["name"]
            rest_a = em_all[nm][:, st["first"]:, sl].reshape(N, -1)
            if rest_a.shape[1] == 0:
                rest_a = np.zeros((N, BL))
            im["em" + nm] = np.ascontiguousarray(rest_a.astype(BF))
        in_maps.append(im)

    if "nc" not in _compiled:
        _compiled["nc"] = _build_nc()
    nc = _compiled["nc"]

    res = run_bass_kernel_spmd(nc, in_maps, core_ids=list(range(NCORES)))
    LAST_RESULTS = res

    # --- host postprocessing: stitch shipped boundary states ---
    sh = np.concatenate(
        [res.results[c]["ships"].astype(np.float64) for c in range(NCORES)]
        , axis=0).reshape(NCORES, N, NSHIP * BL)
    S = [np.concatenate([sh[c][:, i * BL:(i + 1) * BL]
                         for c in range(NCORES)], axis=1)
         for i in range(NSHIP)]                                  # each [N,B]
    f1b, f2a, f2b, f3a, f3b, b1b, b2a, b2b, b3a, b3b = S

    def ratio(a, b):
        return (a * b).sum(axis=0) / (b * b).sum(axis=0)

    s3 = ratio(f1b, f2a) * ratio(f2b, f3a)                       # [B]
    u3 = ratio(b1b, b2a) * ratio(b2b, b3a)                       # [B]
    Q = M64 @ b3b                                                # [N,B]
    dot = (f3b * Q).sum(axis=0)
    logZ = (np.log(dot) + np.log(s3) + np.log(u3) + D[ends]).sum()

    # gold score (f64, mirrors reference)
    tb = np.arange(B)
    emit_sc = np.take_along_axis(e64, target[:, :, None].astype(np.int64),
                                 axis=2)[..., 0]
    trans_sc = trans.astype(np.float64)[target[:-1], target[1:]]
    scores = emit_sc.copy()
    scores[1:] += trans_sc
    score = np.where(mask, scores, 0.0).sum()
    score += strans.astype(np.float64)[target[0]].sum()
    score += etrans.astype(np.float64)[target[ends, tb]].sum()

    loss = (logZ - score) / B
    return np.float32(loss)


# revision 28
# speedup vs baseline: 1.5656x; 1.0735x over previous
"""CRF NLL loss kernel for Trainium2 (8 NeuronCores, data-parallel over batch).

Math: the forward recurrence alpha_t = LSE_j(alpha_{t-1,j} + trans[j,k]) + emit_t
is computed in probability space:  P_t = Eemit_t * (Etrans^T @ P_{t-1})
with per-step normalizers d_t = mean_b LSE_k(emit[t,b,:]) (host-precomputed).

Parallel-segment decomposition: products of positive matrices contract in the
Hilbert projective metric (diagonal emission scalings are isometries), so a
chain started from an arbitrary positive vector converges to the true state's
DIRECTION within ~16 steps; magnitudes differ by one scalar per column which
the host recovers by stitching shipped boundary states.  This turns the
T-step serial recurrence into 6 concurrent latency-bound streams:
  forward:  F1 (t 1..109), F2 (94..203, 16-step burn-in), F3 (188..255, burn)
  backward: B1 (t 510..383), B2 (399..319, burn), B3 (335..256, burn)
Backward runs in X-space X_t = Ehat_t * (M @ X_{t+1}); variable sequence ends
are emission-rewrites on the host: padded steps hold the Perron vector r of
M = exp(trans) (scalar 1/lambda per step) and the single boundary step uses
v/r, v = M^{-1} exp(etrans), mapping r -> w exactly.
Engine placement: F1,F2,B1 elementwise on DVE (551ns/step chain latency);
F3,B2,B3 use Act-copy(PSUM->SBUF)+GPSIMD-mult (884ns/step) since the
Activation engine has no tensor_tensor and GPSIMD cannot read PSUM.
Host computes logZ_b = log(P(255) . M X(256)) + stitch scalars + D[end_b];
the gold-path score is pure gather work, done on host in f64.
"""

import numpy as np
import ml_dtypes

import concourse.bacc as bacc
import concourse.mybir as mybir
import concourse.tile as tile
from concourse.bass_utils import run_bass_kernel_spmd

T, B, N = 512, 256, 128
NCORES = 8
BL = B // NCORES          # 32 sequences per core
K = 16                    # burn-in steps for non-initial streams
M1 = 136                  # forward boundary
N1 = 375                  # backward boundary
CHUNK = 32                # emission steps per DMA chunk
F0 = 8                    # first-chunk steps folded into init DMA (F1/B1)

BF = ml_dtypes.bfloat16

# 4 homogeneous DVE streams at the DVE-throughput period (4 x 158.3 ~ 633):
# a uniform engine class avoids the in-order-queue resonance that mixed
# 551/884 chain classes lock into.
def _mk_streams():
    f1 = list(range(1, M1 + 1))
    f2 = list(range(M1 - K + 1, 256))
    b1 = list(range(510, N1 - 1, -1))
    b2 = list(range(N1 + K, 255, -1))
    return [
        dict(name="F1", eng="dve", dr="F", ts=f1, ships={len(f1) - 1: 0},
             init="dma", first=F0, period=633, phase=0),
        dict(name="F2", eng="dve", dr="F", ts=f2,
             ships={K - 1: 1, len(f2) - 1: 2}, init="ones", first=K,
             period=633, phase=158),
        dict(name="B1", eng="dve", dr="B", ts=b1, ships={len(b1) - 1: 3},
             init="dma", first=F0, period=633, phase=316),
        dict(name="B2", eng="dve", dr="B", ts=b2,
             ships={K: 4, len(b2) - 1: 5}, init="ones", first=K,
             period=633, phase=474),
    ]

STREAMS = _mk_streams()
NSHIP = 6

# Optional {(stream, k): vtime} emission-order override (self-consistent
# schedule measured from a TimelineSim run; see EMIT_ORDER below).
SCHEDULE = None

LAST_RESULTS = None       # BassKernelResults of the last run (for profiling)

_compiled = {}


def _build_nc():
    nc = bacc.Bacc("TRN2", target_bir_lowering=False, debug=False,
                   num_devices=NCORES)
    f32 = mybir.dt.float32
    bf16 = mybir.dt.bfloat16

    # merged critical loads: [state0 | weights | first chunks of same-direction
    # streams] -> one HWDGE generation slot each (the generator is shared
    # across queues and serializes at ~630ns)
    IWF = BL + N + F0 * BL + K * BL
    initf = nc.dram_tensor("initf", [N, IWF], bf16, kind="ExternalInput")
    initb = nc.dram_tensor("initb", [N, IWF], bf16, kind="ExternalInput")
    emd = {}
    for st in STREAMS:
        w = (len(st["ts"]) - st["first"]) * BL
        emd[st["name"]] = nc.dram_tensor("em" + st["name"], [N, max(w, BL)],
                                         bf16, kind="ExternalInput")
    ships = nc.dram_tensor("ships", [N, NSHIP * BL], bf16,
                           kind="ExternalOutput")

    with tile.TileContext(nc) as tc:
        with (
            tc.tile_pool(name="const", bufs=1) as cpool,
            tc.tile_pool(name="emit", bufs=max(
                (len(st["ts"]) - st["first"] + CHUNK - 1) // CHUNK
                for st in STREAMS)) as epool,
            tc.tile_pool(name="state", bufs=3) as spool,
            tc.tile_pool(name="tmp", bufs=3) as tpool,
            tc.tile_pool(name="ps", bufs=1, space="PSUM") as pspool,
        ):
            tF0 = cpool.tile([N, IWF], bf16, tag="initf")
            nc.sync.dma_start(tF0[:], initf[:])
            tB0 = cpool.tile([N, IWF], bf16, tag="initb")
            nc.gpsimd.dma_start(tB0[:], initb[:])

            mF = tF0[:, BL:BL + N]
            mB = tB0[:, BL:BL + N]

            # ones init for burn-in streams
            t_ones = cpool.tile([N, BL], bf16, tag="ones")
            nc.vector.memset(t_ones[:], 1.0)

            # per-stream runtime state
        # layout of first-chunk region inside init tiles:
            #   [state0 | M | first(F1 or B1) | first(F2/B2) | first(F3/B3)]
            offs = {
                "F1": BL + N, "B1": BL + N,
                "F2": BL + N + F0 * BL, "B2": BL + N + F0 * BL,
            }
            rt = {}
            chunk_reqs = []   # (need_vtime, stream, chunk lo, hi)
            for st in STREAMS:
                nm = st["name"]
                it = tF0 if st["dr"] == "F" else tB0
                n_steps = len(st["ts"])
                # emission AP per step (first chunk lives in the init tile)
                em_ap = []
                for k in range(st["first"]):
                    em_ap.append((it, offs[nm] + k * BL))
                n_rest = n_steps - st["first"]
                n_ch = (n_rest + CHUNK - 1) // CHUNK
                for c in range(n_ch):
                    lo = c * CHUNK * BL
                    hi = min(n_rest, (c + 1) * CHUNK) * BL
                    need = st["phase"] + (st["first"] + c * CHUNK) * st["period"]
                    chunk_reqs.append((need, nm, lo, hi))
                cur = it[:, 0:BL] if st["init"] == "dma" else t_ones[:]
                rt[nm] = dict(st=st, cur=cur, em=em_ap,
                              m=mF if st["dr"] == "F" else mB)

            # all chunk DMAs on SP, ordered by first-consumption time, so the
            # Activation sequencer serves only the hybrid-stream copies
            chunk_reqs.sort()
            for _, nm, lo, hi in chunk_reqs:
                ch_t = epool.tile([N, CHUNK * BL], bf16, tag="em" + nm)
                nc.sync.dma_start(ch_t[:, :hi - lo], emd[nm][:, lo:hi])
                for k in range((hi - lo) // BL):
                    rt[nm]["em"].append((ch_t, k * BL))

            # ship destination tiles (dedicated, never recycled)
            ship_t = []
            for i in range(NSHIP):
                sh = cpool.tile([N, BL], bf16, tag=f"ship{i}")
                ship_t.append(sh)

            # static virtual-time schedule
            events = []
            for st in STREAMS:
                for k in range(len(st["ts"])):
                    vt = None
                    if SCHEDULE is not None:
                        vt = SCHEDULE.get((st["name"], k))
                    if vt is None:
                        vt = st["phase"] + k * st["period"]
                    events.append((vt, st["name"], k))
            events.sort()

            for _, nm, k in events:
                r = rt[nm]
                st = r["st"]
                ps_t = pspool.tile([N, BL], f32, tag="ps" + nm)
                nc.tensor.matmul(ps_t[:], r["m"], r["cur"],
                                 start=True, stop=True)
                ch_t, off = r["em"][k]
                em = ch_t[:, off:off + BL]
                slot = st["ships"].get(k)
                if slot is None:
                    o_t = spool.tile([N, BL], bf16, tag="s" + nm)
                    out = o_t[:]
                else:
                    out = ship_t[slot][:]
                if st["eng"] == "dve":
                    nc.vector.tensor_tensor(out, ps_t[:], em,
                                            mybir.AluOpType.mult)
                else:
                    tm_t = tpool.tile([N, BL], bf16, tag="t" + nm)
                    nc.scalar.copy(tm_t[:], ps_t[:])
                    nc.gpsimd.tensor_tensor(out, tm_t[:], em,
                                            mybir.AluOpType.mult)
                r["cur"] = out
                if slot is not None:
                    q = nc.sync if st["dr"] == "F" else nc.scalar
                    q.dma_start(ships[:, slot * BL:(slot + 1) * BL], out)
    nc.compile()
    return nc


def kernel(emit, target, mask, trans, strans, etrans):
    global LAST_RESULTS
    emit = np.asarray(emit, dtype=np.float32)
    target = np.asarray(target, dtype=np.int32)
    mask = np.asarray(mask)
    trans = np.asarray(trans, dtype=np.float32)
    strans = np.asarray(strans, dtype=np.float32)
    etrans = np.asarray(etrans, dtype=np.float32)

    # --- host preprocessing ---
    e64 = emit.astype(np.float64)
    m_t = e64.max(axis=2, keepdims=True)
    lse = (m_t[..., 0] + np.log(np.exp(e64 - m_t).sum(axis=2)))  # [T,B]
    d = lse.mean(axis=1)
    d[0] = 0.0
    D = np.cumsum(d)

    eemn = np.exp(e64 - d[:, None, None])                        # [T,B,N]
    M64 = np.exp(trans.astype(np.float64))                       # [N,N] (j,k)
    w64 = np.exp(etrans.astype(np.float64))

    r = np.ones(N, dtype=np.float64)
    for _ in range(60):
        r = M64 @ r
        r /= r.mean()
    lam = float((r @ (M64 @ r)) / (r @ r))
    v = np.linalg.solve(M64, w64)

    L = mask.astype(np.int64).sum(axis=0)
    ends = L - 1

    P0 = np.exp(strans[None, :].astype(np.float64) + e64[0]).T   # [N,B]

    # backward emissions Ehat_t for t=256..510 indexed [N, t, B]
    tt = np.arange(256, 511)
    EB = eemn[256:511].transpose(2, 0, 1).copy()                 # [N,255,B]
    pad = (tt[None, :] > L[:, None]).T[None, :, :]
    bnd = (tt[None, :] == L[:, None]).T[None, :, :]
    EB = np.where(pad, 1.0 / lam, EB)
    EB = np.where(bnd, (v / r)[:, None, None], EB)

    def em_at(t):
        # [N, B] emission consumed at step t (fwd E'_t or bwd Ehat_t)
        if t <= 255:
            return eemn[t].T
        return EB[:, t - 256, :]

    X0 = np.empty((N, B), dtype=np.float64)
    full = L == T
    last = L == T - 1
    rest = ~(full | last)
    if full.any():
        X0[:, full] = (eemn[511, full, :] * w64[None, :]).T
    if last.any():
        X0[:, last] = v[:, None]
    if rest.any():
        X0[:, rest] = (r / lam)[:, None]

    # per-stream emission arrays in consumption order
    em_all = {}
    for st in STREAMS:
        em_all[st["name"]] = np.stack([em_at(t) for t in st["ts"]],
                                      axis=1)                    # [N,steps,B]

    in_maps = []
    Mbf = M64.astype(BF)
    MTbf = np.ascontiguousarray(M64.T).astype(BF)
    for c in range(NCORES):
        sl = slice(c * BL, (c + 1) * BL)
        im = {}
        for drn, s0, mm, first_sts in (
                ("initf", P0[:, sl], Mbf, ("F1", "F2")),
                ("initb", X0[:, sl], MTbf, ("B1", "B2"))):
            parts = [s0.astype(BF), mm]
            for nm in first_sts:
                st = next(s for s in STREAMS if s["name"] == nm)
                parts.append(em_all[nm][:, :st["first"], sl]
                             .reshape(N, -1).astype(BF))
            im[drn] = np.ascontiguousarray(np.concatenate(parts, axis=1))
        for st in STREAMS:
            nm = st["name"]
            rest_a = em_all[nm][:, st["first"]:, sl].reshape(N, -1)
            if rest_a.shape[1] == 0:
                rest_a = np.zeros((N, BL))
            im["em" + nm] = np.ascontiguousarray(rest_a.astype(BF))
        in_maps.append(im)

    if "nc" not in _compiled:
        _compiled["nc"] = _build_nc()
    nc = _compiled["nc"]

    res = run_bass_kernel_spmd(nc, in_maps, core_ids=list(range(NCORES)))
    LAST_RESULTS = res

    # --- host postprocessing: stitch shipped boundary states ---
    sh = np.concatenate(
        [res.results[c]["ships"].astype(np.float64) for c in range(NCORES)]
        , axis=0).reshape(NCORES, N, NSHIP * BL)
    S = [np.concatenate([sh[c][:, i * BL:(i + 1) * BL]
                         for c in range(NCORES)], axis=1)
         for i in range(NSHIP)]                                  # each [N,B]
    f1b, f2a, f2b, b1b, b2a, b2b = S

    def ratio(a, b):
        return (a * b).sum(axis=0) / (b * b).sum(axis=0)

    s2 = ratio(f1b, f2a)                                         # [B]
    u2 = ratio(b1b, b2a)                                         # [B]
    Q = M64 @ b2b                                                # [N,B]
    dot = (f2b * Q).sum(axis=0)
    logZ = (np.log(dot) + np.log(s2) + np.log(u2) + D[ends]).sum()

    # gold score (f64, mirrors reference)
    tb = np.arange(B)
    emit_sc = np.take_along_axis(e64, target[:, :, None].astype(np.int64),
                                 axis=2)[..., 0]
    trans_sc = trans.astype(np.float64)[target[:-1], target[1:]]
    scores = emit_sc.copy()
    scores[1:] += trans_sc
    score = np.where(mask, scores, 0.0).sum()
    score += strans.astype(np.float64)[target[0]].sum()
    score += etrans.astype(np.float64)[target[ends, tb]].sum()

    loss = (logZ - score) / B
    return np.float32(loss)
